# revision 22
# baseline (speedup 1.0000x reference)
"""Trainium2 Bass kernel for nn_BinaryEncoding (per-position top-16 mask
along the 256-filter dim of [32, 256, 56, 56] activations).

Pipeline per [128pos x 256ch] block (x transposed onto PSUM by PE):
  m1 = max8(x)                      # DVE, t8 = m1[:,7]
  v  = match_replace(x, m1, -2^100) # DVE: remove top-8 -> SBUF
  m2 = max8(v)                      # DVE, t16 = m2[:,7] = exact x_(16)
  bias = -1e20*t16 + 1e14           # two tiny [128,1] Pool TT ops
  out  = Relu(x*1e20 + bias)        # ScalarE from PSUM -> uint8 saturates
                                    #   to {0,255}; 255 <=> x >= t16 - 1e-6
The mask leaves the device as uint8 in [partition, block, ch] layout; the
host transposes back to [img, ch, pos] and maps raw!=0 -> 1.0f (exact).
vs the baseline: no second match_replace, no output transposes on PE, no
PSUM round-trip for the mask, and 4x less output DMA.

Correctness: threshold t16 - 1e-6 misclassifies only if the fp32 gap
x_(16)-x_(17) < 1e-6 (~1e-4 of positions -> a handful of elements vs
the 2e-2 rel-err budget; verified in numpy).

Sharding: pure data parallel, 4 images per core across 8 cores.
"""

import numpy as np

import concourse.bacc as bacc
import concourse.bass as bass
import concourse.mybir as mybir
from concourse import tile
from concourse.alu_op_type import AluOpType
from concourse.bass_utils import run_bass_kernel_spmd
from concourse.masks import make_identity

P = 128
C = 256                      # filter dim
N_CORES = 8
SENT = -(2.0 ** 100)         # match_replace sentinel
BIGS = 1.0e20                # mask scale: Relu(x*BIGS - BIGS*(t16-EPS))
EPS = 1.0e-6                 # include x_(16) under >= via t16-EPS threshold


def _segments(s, e, hw):
    """Split flat-position range [s, e) into per-image contiguous pieces.

    Returns [(img, h0, h1, off)] with off the offset inside the chunk."""
    res = []
    off = 0
    while s < e:
        img = s // hw
        h0 = s - img * hw
        h1 = min(e - img * hw, hw)
        res.append((img, h0, h1, off))
        off += h1 - h0
        s = img * hw + h1
    return res


def _strip_self_waits(nc, engines=("DVE",)):
    """Remove semaphore waits where an instruction waits on its OWN
    engine's semaphore (engines run their stream in order, so these are
    always satisfied and only add a sem round-trip per dispatch)."""
    n = 0
    for blk in nc.m.functions[0].blocks:
        for inst in blk.instructions:
            eng = str(getattr(inst, "engine", ""))
            si = getattr(inst, "sync_info", None)
            if si is None or not si.on_wait:
                continue
            eng_name = eng.split(".")[-1]
            if eng_name not in engines:
                continue
            keep = [w for w in si.on_wait
                    if not (w.ant_name or "").startswith(eng_name + "_")]
            if len(keep) != len(si.on_wait):
                n += len(si.on_wait) - len(keep)
                si.on_wait = keep
    return n


def build_nc(n_img=4, hw=3136, chunk_blocks=14, in_bufs=3, out_bufs=3,
             v_bufs=6, ps_bufs=4, taper=True, strip_self_waits=(),
             bias_engine="gpsimd", use_stt=False, pipe2=False):
    tot = n_img * hw
    assert tot % P == 0
    nblk = tot // P
    if taper == 4 and nblk >= 24:
        # taper3 head + 1-block final chunk (minimal end-of-stream drain)
        plan = [4, 8]
        while sum(plan) + chunk_blocks <= nblk - 7:
            plan.append(chunk_blocks)
        rem = nblk - sum(plan)
        if rem > 5:
            plan.extend([rem - 3, 2, 1])
        elif rem > 0:
            plan.append(rem)
    elif taper == 3 and nblk >= 24:
        # bigger head chunks: chunk0 must cover chunk1's DMA latency
        # (a [2,4] head leaves a ~2.4us DVE hole waiting for chunk 1)
        plan = [4, 8]
        while sum(plan) + chunk_blocks <= nblk - 6:
            plan.append(chunk_blocks)
        rem = nblk - sum(plan)
        if rem > 4:
            plan.extend([rem - 2, 2])
        elif rem > 0:
            plan.append(rem)
    elif taper == 2 and nblk >= 24:
        # head-only taper: tiny leading chunks so the first max8 starts
        # asap; no tail taper (it added per-chunk overhead mid-stream)
        plan = [1, 2, 4]
        while sum(plan) + chunk_blocks <= nblk - 2:
            plan.append(chunk_blocks)
        rem = nblk - sum(plan)
        if rem > 0:
            plan.append(rem)
    elif taper and nblk >= 24:
        # small first/last chunks shrink the DMA ramp at kernel start/end
        plan = [2, 4]
        while sum(plan) + chunk_blocks <= nblk - 6:
            plan.append(chunk_blocks)
        rem = nblk - sum(plan)
        if rem > 4:
            plan.extend([rem - 2, 2])
        elif rem > 0:
            plan.append(rem)
    else:
        assert nblk % chunk_blocks == 0
        plan = [chunk_blocks] * (nblk // chunk_blocks)
    assert sum(plan) == nblk
    f32 = mybir.dt.float32
    u8 = mybir.dt.uint8

    nc = bacc.Bacc("TRN2", target_bir_lowering=False, debug=False,
                   num_devices=N_CORES)
    x = nc.declare_dram_parameter("x", [n_img, C, hw], f32, isOutput=False)
    # Output in device block layout: y2[p, blk, c] = mask at flat position
    # blk*128+p (across the core's images), channel c.
    y2 = nc.declare_dram_parameter("y2", [P, nblk, C], u8, isOutput=True)

    with tile.TileContext(nc) as tc:
        with (
            tc.tile_pool(name="const", bufs=1) as const_pool,
            tc.tile_pool(name="inp", bufs=in_bufs) as in_pool,
            tc.tile_pool(name="outp", bufs=out_bufs) as out_pool,
            tc.tile_pool(name="vv", bufs=v_bufs) as v_pool,
            tc.tile_pool(name="m8", bufs=2 * v_bufs) as m_pool,
            tc.tile_pool(name="bias", bufs=2 * v_bufs) as b_pool,
            tc.tile_pool(name="psin", bufs=ps_bufs, space="PSUM") as psin_pool,
        ):
            ident = const_pool.tile([P, P], f32)
            make_identity(nc, ident)
            c_negbig = const_pool.tile([P, 1], f32, tag="c_negbig")
            nc.gpsimd.memset(c_negbig, -BIGS)
            c_eps = const_pool.tile([P, 1], f32, tag="c_eps")
            nc.gpsimd.memset(c_eps, BIGS * EPS)
            b_eng = getattr(nc, bias_engine)

            blk0 = 0
            for cb in plan:
                s = blk0 * P
                Lc = cb * P
                segs = _segments(s, s + Lc, hw)

                in_lo = in_pool.tile([P, Lc], f32, tag="in_lo")
                in_hi = in_pool.tile([P, Lc], f32, tag="in_hi")
                if blk0 == 0 and taper == 4:
                    # per-block DMAs: the first transpose starts after one
                    # 128-pos slice lands, not the whole chunk
                    for b in range(cb):
                        bs = b * P
                        nc.sync.dma_start(out=in_lo[:, bs:bs + P],
                                          in_=x[0, 0:P, bs:bs + P])
                        nc.sync.dma_start(out=in_hi[:, bs:bs + P],
                                          in_=x[0, P:C, bs:bs + P])
                else:
                    for (img, h0, h1, off) in segs:
                        n = h1 - h0
                        nc.sync.dma_start(out=in_lo[:, off:off + n],
                                          in_=x[img, 0:P, h0:h1])
                        nc.sync.dma_start(out=in_hi[:, off:off + n],
                                          in_=x[img, P:C, h0:h1])

                outc = out_pool.tile([P, cb, C], u8, tag="outc")

                state = {}

                def emit_front(b):
                    sl = slice(b * P, (b + 1) * P)
                    ps_in = psin_pool.tile([P, C], f32, tag="ps_in",
                                           name="ps_in")
                    nc.tensor.transpose(ps_in[:, 0:P], in_lo[:, sl], ident)
                    nc.tensor.transpose(ps_in[:, P:C], in_hi[:, sl], ident)
                    m1 = m_pool.tile([P, 8], f32, tag="m1", name="m1")
                    state[b] = [ps_in, m1, None, None]
                    nc.vector.max(out=m1, in_=ps_in)

                def emit_mid(b):
                    ps_in, m1, _, _ = state[b]
                    v = v_pool.tile([P, C], f32, tag="v", name="v")
                    m2 = m_pool.tile([P, 8], f32, tag="m2", name="m2")
                    state[b][2] = v
                    state[b][3] = m2
                    if use_stt:
                        # v = (x < t8) * x  (zeros replace the top-8; the
                        # 16th largest stays > 0 for 256 normals)
                        nc.vector.scalar_tensor_tensor(
                            out=v, in0=ps_in, scalar=m1[:, 7:8], in1=ps_in,
                            op0=AluOpType.is_lt, op1=AluOpType.mult)
                    else:
                        nc.vector.match_replace(out=v, in_to_replace=m1,
                                                in_values=ps_in,
                                                imm_value=SENT)

                def emit_back(b):
                    ps_in, m1, v, m2 = state.pop(b)
                    nc.vector.max(out=m2, in_=v)
                    # bias = -BIGS*t16 + BIGS*EPS, per-partition [128,1]
                    bias = b_pool.tile([P, 1], f32, tag="bias", name="bias")
                    if bias_engine == "scalar":
                        # one affine op: Copy(t16 * -BIGS + BIGS*EPS)
                        nc.scalar.activation(
                            bias, m2[:, 7:8],
                            mybir.ActivationFunctionType.Copy,
                            bias=BIGS * EPS, scale=-BIGS)
                    else:
                        b_eng.tensor_tensor(out=bias, in0=m2[:, 7:8],
                                            in1=c_negbig, op=AluOpType.mult)
                        b_eng.tensor_tensor(out=bias, in0=bias, in1=c_eps,
                                            op=AluOpType.add)
                    # uint8 saturation: 0 below t16-EPS, 255 at/above
                    nc.scalar.activation(outc[:, b, :], ps_in,
                                         mybir.ActivationFunctionType.Relu,
                                         bias=bias, scale=BIGS)

                if pipe2:
                    # 2-block stagger: every DVE instruction's producer is
                    # ~3 DVE instructions older, hiding result-forwarding
                    # stalls between the dependent m1 -> v -> m2 chain.
                    for b in range(cb):
                        emit_front(b)
                        if b >= 1:
                            emit_mid(b - 1)
                        if b >= 2:
                            emit_back(b - 2)
                    emit_mid(cb - 1)
                    if cb >= 2:
                        emit_back(cb - 2)
                    emit_back(cb - 1)
                else:
                    for b in range(cb):
                        emit_front(b)
                        emit_mid(b)
                        emit_back(b)

                nc.sync.dma_start(out=y2[:, blk0:blk0 + cb, :], in_=outc)
                blk0 += cb
    nc.compile()
    if strip_self_waits:
        _strip_self_waits(nc, tuple(strip_self_waits))
    return nc


def _install_neff_cache():
    """Cache compiled NEFFs by BIR hash under /tmp so repeat runs skip
    the multi-minute neuronxcc compile."""
    import hashlib
    import os
    import shutil
    import concourse.bass2jax as b2j
    if getattr(b2j, "_topk_neff_cache_installed", False):
        return
    cache_dir = "/tmp/neff_cache"
    try:
        os.makedirs(cache_dir, exist_ok=True)
    except OSError:
        return
    orig_compile = b2j.compile_bir_kernel

    def cached_compile(ant_bir_str, compile_dir_path, neff_name):
        key = hashlib.sha256(ant_bir_str).hexdigest()[:32]
        cpath = os.path.join(cache_dir, key + ".neff")
        if os.path.exists(cpath):
            dst = os.path.join(compile_dir_path, neff_name)
            shutil.copy(cpath, dst)
            return dst
        out = orig_compile(ant_bir_str, compile_dir_path, neff_name=neff_name)
        try:
            shutil.copy(out, cpath)
        except OSError:
            pass
        return out

    b2j.compile_bir_kernel = cached_compile
    b2j._topk_neff_cache_installed = True


_install_neff_cache()

_NC_CACHE = {}


def _get_nc(n_img, hw, chunk_blocks, **kw):
    key = (n_img, hw, chunk_blocks, tuple(sorted(kw.items())))
    if key not in _NC_CACHE:
        _NC_CACHE[key] = build_nc(n_img, hw, chunk_blocks, **kw)
    return _NC_CACHE[key]


KERNEL_KW = dict(pipe2=True, ps_bufs=8, taper=4)


def make_in_maps(x, n_img, kw=KERNEL_KW):
    return [{"x": np.ascontiguousarray(x[i * n_img:(i + 1) * n_img])}
            for i in range(N_CORES)]


def unpack_out(res, n_img, hw):
    """y2 [P, nblk, C] uint8 per core -> [B, C, hw] float32."""
    parts = []
    for i in range(N_CORES):
        arr = res.results[i]["y2"]            # [128, nblk, 256]
        a = np.transpose(arr, (2, 1, 0))      # [256, nblk, 128]
        a = (a.reshape(C, n_img, hw) != 0)    # flat pos = blk*128 + p
        parts.append(np.moveaxis(a, 1, 0))    # [n_img, 256, hw]
    return np.concatenate(parts, axis=0).astype(np.float32)


def kernel(activations: np.ndarray) -> np.ndarray:
    B, Cin, H, W = activations.shape
    assert (B, Cin, H, W) == (32, 256, 56, 56)
    hw = H * W
    n_img = B // N_CORES
    x = np.ascontiguousarray(activations, dtype=np.float32).reshape(B, Cin, hw)
    nc = _get_nc(n_img, hw, 14, **KERNEL_KW)
    in_maps = make_in_maps(x, n_img)
    res = run_bass_kernel_spmd(nc, in_maps, list(range(N_CORES)))
    return unpack_out(res, n_img, hw).reshape(B, Cin, H, W)


# revision 23
# speedup vs baseline: 1.2014x; 1.2014x over previous
"""Trainium2 Bass kernel for nn_BinaryEncoding (per-position top-16 mask
along the 256-filter dim of [32, 256, 56, 56] activations).

Pipeline per [128pos x 256ch] block (x transposed onto PSUM by PE):
  m1 = max8(x)                      # DVE, t8 = m1[:,7]
  v  = match_replace(x, m1, -2^100) # DVE: remove top-8 -> SBUF
  m2 = max8(v)                      # DVE, t16 = m2[:,7] = exact x_(16)
  bias = -1e20*t16 + 1e14           # two tiny [128,1] Pool TT ops
  out  = Relu(x*1e20 + bias)        # ScalarE from PSUM -> uint8 saturates
                                    #   to {0,255}; 255 <=> x >= t16 - 1e-6
The mask leaves the device as uint8 in [partition, block, ch] layout; the
host transposes back to [img, ch, pos] and maps raw!=0 -> 1.0f (exact).
vs the baseline: no second match_replace, no output transposes on PE, no
PSUM round-trip for the mask, and 4x less output DMA.

Correctness: threshold t16 - 1e-6 misclassifies only if the fp32 gap
x_(16)-x_(17) < 1e-6 (~1e-4 of positions -> a handful of elements vs
the 2e-2 rel-err budget; verified in numpy).

Sharding: pure data parallel, 4 images per core across 8 cores.
"""

import numpy as np

import concourse.bacc as bacc
import concourse.bass as bass
import concourse.mybir as mybir
from concourse import tile
from concourse.alu_op_type import AluOpType
from concourse.bass_utils import run_bass_kernel_spmd
from concourse.masks import make_identity

P = 128
C = 256                      # filter dim
N_CORES = 8
SENT = -(2.0 ** 100)         # match_replace sentinel
BIGS = 1.0e20                # mask scale: Relu(x*BIGS - BIGS*(t16-EPS))
EPS = 1.0e-6                 # include x_(16) under >= via t16-EPS threshold


def _segments(s, e, hw):
    """Split flat-position range [s, e) into per-image contiguous pieces.

    Returns [(img, h0, h1, off)] with off the offset inside the chunk."""
    res = []
    off = 0
    while s < e:
        img = s // hw
        h0 = s - img * hw
        h1 = min(e - img * hw, hw)
        res.append((img, h0, h1, off))
        off += h1 - h0
        s = img * hw + h1
    return res


def _strip_self_waits(nc, engines=("DVE",)):
    """Remove semaphore waits where an instruction waits on its OWN
    engine's semaphore (engines run their stream in order, so these are
    always satisfied and only add a sem round-trip per dispatch)."""
    n = 0
    for blk in nc.m.functions[0].blocks:
        for inst in blk.instructions:
            eng = str(getattr(inst, "engine", ""))
            si = getattr(inst, "sync_info", None)
            if si is None or not si.on_wait:
                continue
            eng_name = eng.split(".")[-1]
            if eng_name not in engines:
                continue
            keep = [w for w in si.on_wait
                    if not (w.ant_name or "").startswith(eng_name + "_")]
            if len(keep) != len(si.on_wait):
                n += len(si.on_wait) - len(keep)
                si.on_wait = keep
    return n


def build_nc(n_img=4, hw=3136, chunk_blocks=14, in_bufs=3, out_bufs=3,
             v_bufs=6, ps_bufs=4, taper=True, strip_self_waits=(),
             bias_engine="gpsimd", use_stt=False, pipe2=False):
    tot = n_img * hw
    assert tot % P == 0
    nblk = tot // P
    if taper == 4 and nblk >= 24:
        # taper3 head + 1-block final chunk (minimal end-of-stream drain)
        plan = [4, 8]
        while sum(plan) + chunk_blocks <= nblk - 7:
            plan.append(chunk_blocks)
        rem = nblk - sum(plan)
        if rem > 5:
            plan.extend([rem - 3, 2, 1])
        elif rem > 0:
            plan.append(rem)
    elif taper == 3 and nblk >= 24:
        # bigger head chunks: chunk0 must cover chunk1's DMA latency
        # (a [2,4] head leaves a ~2.4us DVE hole waiting for chunk 1)
        plan = [4, 8]
        while sum(plan) + chunk_blocks <= nblk - 6:
            plan.append(chunk_blocks)
        rem = nblk - sum(plan)
        if rem > 4:
            plan.extend([rem - 2, 2])
        elif rem > 0:
            plan.append(rem)
    elif taper == 2 and nblk >= 24:
        # head-only taper: tiny leading chunks so the first max8 starts
        # asap; no tail taper (it added per-chunk overhead mid-stream)
        plan = [1, 2, 4]
        while sum(plan) + chunk_blocks <= nblk - 2:
            plan.append(chunk_blocks)
        rem = nblk - sum(plan)
        if rem > 0:
            plan.append(rem)
    elif taper and nblk >= 24:
        # small first/last chunks shrink the DMA ramp at kernel start/end
        plan = [2, 4]
        while sum(plan) + chunk_blocks <= nblk - 6:
            plan.append(chunk_blocks)
        rem = nblk - sum(plan)
        if rem > 4:
            plan.extend([rem - 2, 2])
        elif rem > 0:
            plan.append(rem)
    else:
        assert nblk % chunk_blocks == 0
        plan = [chunk_blocks] * (nblk // chunk_blocks)
    assert sum(plan) == nblk
    f32 = mybir.dt.float32
    u8 = mybir.dt.uint8

    nc = bacc.Bacc("TRN2", target_bir_lowering=False, debug=False,
                   num_devices=N_CORES)
    x = nc.declare_dram_parameter("x", [n_img, C, hw], f32, isOutput=False)
    # Output in device block layout: y2[p, blk, c] = mask at flat position
    # blk*128+p (across the core's images), channel c.
    y2 = nc.declare_dram_parameter("y2", [P, nblk, C], u8, isOutput=True)

    with tile.TileContext(nc) as tc:
        with (
            tc.tile_pool(name="const", bufs=1) as const_pool,
            tc.tile_pool(name="inp", bufs=in_bufs) as in_pool,
            tc.tile_pool(name="outp", bufs=out_bufs) as out_pool,
            tc.tile_pool(name="vv", bufs=v_bufs) as v_pool,
            tc.tile_pool(name="m8", bufs=2 * v_bufs) as m_pool,
            tc.tile_pool(name="bias", bufs=2 * v_bufs) as b_pool,
            tc.tile_pool(name="psin", bufs=ps_bufs, space="PSUM") as psin_pool,
        ):
            ident = const_pool.tile([P, P], f32)
            make_identity(nc, ident)
            c_negbig = const_pool.tile([P, 1], f32, tag="c_negbig")
            nc.gpsimd.memset(c_negbig, -BIGS)
            c_eps = const_pool.tile([P, 1], f32, tag="c_eps")
            nc.gpsimd.memset(c_eps, BIGS * EPS)
            b_eng = getattr(nc, bias_engine)

            blk0 = 0
            for cb in plan:
                s = blk0 * P
                Lc = cb * P
                segs = _segments(s, s + Lc, hw)

                in_lo = in_pool.tile([P, Lc], f32, tag="in_lo")
                in_hi = in_pool.tile([P, Lc], f32, tag="in_hi")
                if blk0 == 0 and taper == 4:
                    # per-block DMAs: the first transpose starts after one
                    # 128-pos slice lands, not the whole chunk
                    for b in range(cb):
                        bs = b * P
                        nc.sync.dma_start(out=in_lo[:, bs:bs + P],
                                          in_=x[0, 0:P, bs:bs + P])
                        nc.sync.dma_start(out=in_hi[:, bs:bs + P],
                                          in_=x[0, P:C, bs:bs + P])
                else:
                    for (img, h0, h1, off) in segs:
                        n = h1 - h0
                        nc.sync.dma_start(out=in_lo[:, off:off + n],
                                          in_=x[img, 0:P, h0:h1])
                        nc.sync.dma_start(out=in_hi[:, off:off + n],
                                          in_=x[img, P:C, h0:h1])

                outc = out_pool.tile([P, cb, C], u8, tag="outc")

                state = {}

                def emit_front(b):
                    sl = slice(b * P, (b + 1) * P)
                    ps_in = psin_pool.tile([P, C], f32, tag="ps_in",
                                           name="ps_in")
                    nc.tensor.transpose(ps_in[:, 0:P], in_lo[:, sl], ident)
                    nc.tensor.transpose(ps_in[:, P:C], in_hi[:, sl], ident)
                    m1 = m_pool.tile([P, 8], f32, tag="m1", name="m1")
                    state[b] = [ps_in, m1, None, None]
                    nc.vector.max(out=m1, in_=ps_in)

                def emit_mid(b):
                    ps_in, m1, _, _ = state[b]
                    v = v_pool.tile([P, C], f32, tag="v", name="v")
                    m2 = m_pool.tile([P, 8], f32, tag="m2", name="m2")
                    state[b][2] = v
                    state[b][3] = m2
                    if use_stt:
                        # v = (x < t8) * x  (zeros replace the top-8; the
                        # 16th largest stays > 0 for 256 normals)
                        nc.vector.scalar_tensor_tensor(
                            out=v, in0=ps_in, scalar=m1[:, 7:8], in1=ps_in,
                            op0=AluOpType.is_lt, op1=AluOpType.mult)
                    else:
                        nc.vector.match_replace(out=v, in_to_replace=m1,
                                                in_values=ps_in,
                                                imm_value=SENT)

                def emit_back(b):
                    ps_in, m1, v, m2 = state.pop(b)
                    nc.vector.max(out=m2, in_=v)
                    # bias = -BIGS*t16 + BIGS*EPS, per-partition [128,1]
                    bias = b_pool.tile([P, 1], f32, tag="bias", name="bias")
                    if bias_engine == "scalar":
                        # one affine op: Copy(t16 * -BIGS + BIGS*EPS)
                        nc.scalar.activation(
                            bias, m2[:, 7:8],
                            mybir.ActivationFunctionType.Copy,
                            bias=BIGS * EPS, scale=-BIGS)
                    else:
                        b_eng.tensor_tensor(out=bias, in0=m2[:, 7:8],
                                            in1=c_negbig, op=AluOpType.mult)
                        b_eng.tensor_tensor(out=bias, in0=bias, in1=c_eps,
                                            op=AluOpType.add)
                    # uint8 saturation: 0 below t16-EPS, 255 at/above
                    nc.scalar.activation(outc[:, b, :], ps_in,
                                         mybir.ActivationFunctionType.Relu,
                                         bias=bias, scale=BIGS)

                if pipe2:
                    # 2-block stagger: every DVE instruction's producer is
                    # ~3 DVE instructions older, hiding result-forwarding
                    # stalls between the dependent m1 -> v -> m2 chain.
                    for b in range(cb):
                        emit_front(b)
                        if b >= 1:
                            emit_mid(b - 1)
                        if b >= 2:
                            emit_back(b - 2)
                    emit_mid(cb - 1)
                    if cb >= 2:
                        emit_back(cb - 2)
                    emit_back(cb - 1)
                else:
                    for b in range(cb):
                        emit_front(b)
                        emit_mid(b)
                        emit_back(b)

                nc.sync.dma_start(out=y2[:, blk0:blk0 + cb, :], in_=outc)
                blk0 += cb
    nc.compile()
    if strip_self_waits:
        _strip_self_waits(nc, tuple(strip_self_waits))
    return nc


def _install_neff_cache():
    """Cache compiled NEFFs by BIR hash under /tmp so repeat runs skip
    the multi-minute neuronxcc compile."""
    import hashlib
    import os
    import shutil
    import concourse.bass2jax as b2j
    if getattr(b2j, "_topk_neff_cache_installed", False):
        return
    cache_dir = "/tmp/neff_cache"
    try:
        os.makedirs(cache_dir, exist_ok=True)
    except OSError:
        return
    orig_compile = b2j.compile_bir_kernel

    def cached_compile(ant_bir_str, compile_dir_path, neff_name):
        key = hashlib.sha256(ant_bir_str).hexdigest()[:32]
        cpath = os.path.join(cache_dir, key + ".neff")
        if os.path.exists(cpath):
            dst = os.path.join(compile_dir_path, neff_name)
            shutil.copy(cpath, dst)
            return dst
        out = orig_compile(ant_bir_str, compile_dir_path, neff_name=neff_name)
        try:
            shutil.copy(out, cpath)
        except OSError:
            pass
        return out

    b2j.compile_bir_kernel = cached_compile
    b2j._topk_neff_cache_installed = True


_install_neff_cache()

_NC_CACHE = {}


def _get_nc(n_img, hw, chunk_blocks, **kw):
    key = (n_img, hw, chunk_blocks, tuple(sorted(kw.items())))
    if key not in _NC_CACHE:
        _NC_CACHE[key] = build_nc(n_img, hw, chunk_blocks, **kw)
    return _NC_CACHE[key]


KERNEL_KW = dict(pipe2=True, ps_bufs=8, taper=3)


def make_in_maps(x, n_img, kw=KERNEL_KW):
    return [{"x": np.ascontiguousarray(x[i * n_img:(i + 1) * n_img])}
            for i in range(N_CORES)]


def unpack_out(res, n_img, hw):
    """y2 [P, nblk, C] uint8 per core -> [B, C, hw] float32."""
    parts = []
    for i in range(N_CORES):
        arr = res.results[i]["y2"]            # [128, nblk, 256]
        a = np.transpose(arr, (2, 1, 0))      # [256, nblk, 128]
        a = (a.reshape(C, n_img, hw) != 0)    # flat pos = blk*128 + p
        parts.append(np.moveaxis(a, 1, 0))    # [n_img, 256, hw]
    return np.concatenate(parts, axis=0).astype(np.float32)


def kernel(activations: np.ndarray) -> np.ndarray:
    B, Cin, H, W = activations.shape
    assert (B, Cin, H, W) == (32, 256, 56, 56)
    hw = H * W
    n_img = B // N_CORES
    x = np.ascontiguousarray(activations, dtype=np.float32).reshape(B, Cin, hw)
    nc = _get_nc(n_img, hw, 14, **KERNEL_KW)
    in_maps = make_in_maps(x, n_img)
    res = run_bass_kernel_spmd(nc, in_maps, list(range(N_CORES)))
    return unpack_out(res, n_img, hw).reshape(B, Cin, H, W)
